# revision 23
# baseline (speedup 1.0000x reference)
"""Trainium2 Bass kernel for nn_CausalSelfAttention_10368051052888.

Head-sharded tensor parallel over 8 NeuronCores (2 heads/core).
Feature-major ("transposed") layout on device: activations live as
[feature, seq] so the PE contraction dim is always the partition dim.

v2: software-pipelined per-slice schedule.
  - chunked weight DMAs so the first projection matmul starts ~6us in
  - ssq exchange as 3 small AllGathers (2 slices each) + local reduce,
    replacing the 2 serial latency-bound AllReduces
  - attention for slice j starts as soon as rope(j) is done; remaining
    qk/v projections and yproj fill PE while ACT runs the exp stream
  - one attention AllGather per slice (both heads) instead of per-head
  - per-slice staging tiles (rq/knew/vnew) for fine-grained deps

Self-contained: hardcodes the problem shapes from the spec.
"""
import numpy as np
import ml_dtypes

import concourse.bass as bass
import concourse.bass_isa as bass_isa
import concourse.mybir as mybir
import concourse.tile as tile
from concourse import bacc
from concourse.bass_utils import run_bass_kernel_spmd

BF = ml_dtypes.bfloat16

N_CORES = 8
S = 2640
D = 2048
H = 16
HD = 128
CACHE = 5280
EPS = 1e-6

HPC = H // N_CORES          # heads per core = 2
MF = HPC * HD               # per-core feature slice = 256
L = CACHE + S               # 7920
KC = D // 128               # 16 contraction chunks
WCH = 4                     # weight DMA chunks (4 kc each)
CTILES = (CACHE + 127) // 128   # 42 cache k-tiles (last kt=32)
NTILES = (S + 127) // 128       # 21 new k-tiles / v s-tiles (last 80)
VPAD = CTILES * 128             # 5376 padded cache rows for v
NQ = 512
N_SLICES = [(i * NQ, min(NQ, S - i * NQ)) for i in range((S + NQ - 1) // NQ)]
NJ = len(N_SLICES)
NG = 3                      # ssq exchange groups (2 slices each)
GSZ = 8                     # pairs per bf16 partial-sum group

SWAP_MASK = [(i ^ 1) for i in range(32)]  # pair swap within 32-partition groups

# wo contraction-row permutation: AG chunk layout is (core r, local head m,
# dim d) -> global feature (2r+m)*128+d
AG_PERM = np.array([(2 * (i // MF) + (i % MF) // HD) * HD + i % HD
                    for i in range(D)])

_prog_cache = {}


def build_program():
    dt = mybir.dt
    f32, bf16 = dt.float32, dt.bfloat16
    nc = bacc.Bacc("TRN2", target_bir_lowering=False, debug=False,
                   num_devices=N_CORES)

    # ---------------- I/O ----------------
    xT = nc.dram_tensor("xT", [D, S], bf16, kind="ExternalInput")
    wq = nc.dram_tensor("wq", [WCH, 128, 4 * MF], bf16, kind="ExternalInput")
    wk = nc.dram_tensor("wk", [WCH, 128, 4 * MF], bf16, kind="ExternalInput")
    wv = nc.dram_tensor("wv", [WCH, 128, 4 * MF], bf16, kind="ExternalInput")
    wo = nc.dram_tensor("wo", [WCH, 128, 4 * MF], bf16, kind="ExternalInput")
    cosT = nc.dram_tensor("cosT", [128, S], bf16, kind="ExternalInput")
    sinT = nc.dram_tensor("sinT", [128, S], bf16, kind="ExternalInput")
    kTc = nc.dram_tensor("kTc", [HPC, 128, CACHE], bf16, kind="ExternalInput")
    vc = nc.dram_tensor("vc", [HPC, 128, VPAD], bf16, kind="ExternalInput")
    masks = nc.dram_tensor("masks", [4, 128, NQ], bf16, kind="ExternalInput")
    yT = nc.dram_tensor("yT", [MF, S], f32, kind="ExternalOutput")

    GCOLS = [N_SLICES[2 * g][1] + N_SLICES[2 * g + 1][1] for g in range(NG)]
    ssqg_in = [nc.dram_tensor(f"ssqg_in{g}", [2, GCOLS[g]], f32)
               for g in range(NG)]
    ssqg_out = [nc.dram_tensor(f"ssqg_out{g}", [N_CORES, 2, GCOLS[g]], f32,
                               addr_space="Shared") for g in range(NG)]
    ag_in = [nc.dram_tensor(f"ag_in{j}", [HPC, 128, N_SLICES[j][1]], bf16)
             for j in range(NJ)]
    ag_out = [nc.dram_tensor(f"ag_out{j}",
                             [N_CORES, HPC, 128, N_SLICES[j][1]], bf16,
                             addr_space="Shared") for j in range(NJ)]

    RG = [list(range(N_CORES))]
    Exp = mybir.ActivationFunctionType.Exp
    Sqrt = mybir.ActivationFunctionType.Sqrt
    add_op = mybir.AluOpType.add
    mult_op = mybir.AluOpType.mult

    with tile.TileContext(nc) as tc:
        with (
            tc.tile_pool(name="const", bufs=1) as constp,
            tc.tile_pool(name="xs", bufs=4) as xsp,
            tc.tile_pool(name="work", bufs=3) as workp,
            tc.tile_pool(name="ftmp", bufs=3) as ftmp,
            tc.tile_pool(name="attn", bufs=2) as attnp,
            tc.tile_pool(name="ptp", bufs=3) as ptp,
            tc.tile_pool(name="psac", bufs=4, space="PSUM") as psac,
            tc.tile_pool(name="pssc", bufs=2, space="PSUM") as pssc,
        ):
            # ------------ persistent SBUF tiles ------------
            w_sb = {t: [constp.tile([128, 4 * MF], bf16, tag=f"w{t}{c}",
                                    name=f"w{t}{c}") for c in range(WCH)]
                    for t in ("q", "k", "v", "o")}

            def wslice(t, kc, m):
                c, kcl = kc // 4, kc % 4
                return w_sb[t][c][:, kcl * MF + m * 128:kcl * MF + (m + 1) * 128]

            def wslice_full(t, kc):
                c, kcl = kc // 4, kc % 4
                return w_sb[t][c][:, kcl * MF:(kcl + 1) * MF]

            cos_sb = constp.tile([128, S], bf16, tag="cos")
            sin_sb = constp.tile([128, S], bf16, tag="sin")
            mask_sb = constp.tile([128, 4 * NQ], bf16, tag="masks")
            kTc_sb = [constp.tile([128, CACHE], bf16, tag=f"kTc{h}",
                                  name=f"kTc{h}") for h in range(HPC)]
            vc_sb = [constp.tile([128, VPAD], bf16, tag=f"vc{h}",
                                 name=f"vc{h}") for h in range(HPC)]
            knew = [[constp.tile([128, N_SLICES[j][1]], bf16, tag=f"kn{j}{h}",
                                 name=f"kn{j}{h}") for h in range(HPC)]
                    for j in range(NJ)]
            rq_sb = [[constp.tile([128, N_SLICES[j][1]], bf16, tag=f"rq{j}{h}",
                                  name=f"rq{j}{h}") for h in range(HPC)]
                     for j in range(NJ)]
            vnew = [[constp.tile([128, ((N_SLICES[j][1] + 127) // 128) * 128],
                                 bf16, tag=f"vn{j}{h}",
                                 name=f"vn{j}{h}") for h in range(HPC)]
                    for j in range(NJ)]
            onescol = constp.tile([128, 1], bf16, tag="onescol")
            nc.vector.memset(onescol[:], 1.0)
            eps_col = constp.tile([1, 1], f32, tag="eps")
            nc.vector.memset(eps_col[:], EPS)

            # ------------ prologue DMAs ------------
            # weights chunked and interleaved with the first slice's xs
            # stream so the first projection matmul only waits ~1MB of DMA.
            # Bulk cache/v/mask loads are emitted later, gated behind the
            # first slice's staging so their descriptors can't head-of-line
            # block the xT stream at t=0.
            def bulk_loads():
                # dummy dep: don't start bulk until slice 0 is staged
                gate = workp.tile([128, 1], bf16, tag="gate", bufs=1)
                nc.gpsimd.partition_broadcast(gate[:], rq_sb[0][0][0:1, 0:1])
                for h in range(HPC):
                    nc.gpsimd.dma_start(out=kTc_sb[h][:], in_=kTc[h])
                for h in range(HPC):
                    nc.gpsimd.dma_start(out=vc_sb[h][:], in_=vc[h])
                nc.gpsimd.dma_start(
                    out=mask_sb[:].rearrange("p (d c) -> p d c", c=NQ),
                    in_=masks[:].rearrange("d p c -> p d c"),
                )

            def stream_x(qb, nn, consume, wdma=None):
                """DMA xT[:, qb:qb+nn] in 4-chunk groups; call consume(kc, rhs_ap)."""
                for g in range(KC // 4):
                    if wdma is not None:
                        wdma(g)
                    xs = xsp.tile([128, 4 * NQ], bf16, tag="xs", name="xs")
                    nc.sync.dma_start(
                        out=xs[:].rearrange("p (a n) -> p a n", n=NQ)[:, :, :nn],
                        in_=xT[g * 512:(g + 1) * 512, qb:qb + nn]
                            .rearrange("(a p) n -> p a n", p=128))
                    for kcl in range(4):
                        consume(g * 4 + kcl, xs[:, kcl * NQ:kcl * NQ + nn])

            def qk_proj(j):
                qb, nn = N_SLICES[j]
                pst = {t: [psac.tile([128, NQ], f32, tag="acc",
                                     name=f"proj_{t}{m}")
                           for m in range(HPC)] for t in ("q", "k")}

                def mm_proj(kc, rhs, pst=pst, nn=nn):
                    for t in ("q", "k"):
                        for m in range(HPC):
                            nc.tensor.matmul(
                                pst[t][m][:, :nn], wslice(t, kc, m),
                                rhs, start=(kc == 0), stop=(kc == KC - 1))

                def wdma(g):
                    nc.sync.dma_start(out=w_sb["q"][g][:], in_=wq[g])
                    nc.sync.dma_start(out=w_sb["k"][g][:], in_=wk[g])

                stream_x(qb, nn, mm_proj, wdma=wdma if j == 0 else None)
                # stage raw q/k as bf16 (ACT): rope runs in place later
                for t, dst in (("q", rq_sb[j]), ("k", knew[j])):
                    for m in range(HPC):
                        nc.scalar.copy(dst[m][:, :nn], pst[t][m][:, :nn])

            def ssq_partial(j, part_q, part_k):
                """ssq partial rows for slice j into part_{q,k}[(j%2)*NQ+...]."""
                qb, nn = N_SLICES[j]
                col0 = (j % 2) * NQ
                for src, ptile in ((rq_sb[j], part_q), (knew[j], part_k)):
                    sqp = pssc.tile([128, 2 * NQ], f32, tag="scores",
                                    name="sqp")
                    for m in range(HPC):
                        q2 = workp.tile([128, NQ], bf16, tag="btmp")
                        nc.vector.tensor_tensor(q2[:, :nn], src[m][:, :nn],
                                                src[m][:, :nn], mult_op)
                        nc.tensor.matmul(sqp[:1, :nn], onescol[:],
                                         q2[:, :nn],
                                         start=(m == 0), stop=(m == HPC - 1))
                    nc.scalar.copy(ptile[:, col0:col0 + nn], sqp[:1, :nn])

            def ssq_group(g):
                """Emit partials for slices 2g, 2g+1 and the AllGather."""
                parts = [workp.tile([1, 2 * NQ], f32, tag=f"part{ti}", bufs=1,
                                    name=f"part{g}{ti}") for ti in range(2)]
                ssq_partial(2 * g, parts[0], parts[1])
                ssq_partial(2 * g + 1, parts[0], parts[1])
                ncols = GCOLS[g]
                for ti in range(2):
                    nc.scalar.dma_start(out=ssqg_in[g][ti:ti + 1, :],
                                        in_=parts[ti][:, :ncols])
                nc.gpsimd.collective_compute(
                    "AllGather", mybir.AluOpType.bypass, replica_groups=RG,
                    ins=[ssqg_in[g][:]], outs=[ssqg_out[g][:]])

            srow_g = {}

            def ssq_group_recv(g):
                """Reduce the gathered partials into srow for slices 2g,2g+1."""
                ncols = GCOLS[g]
                srow = [workp.tile([1, 2 * NQ], f32, tag=f"srowg{ti}", bufs=1,
                                   name=f"srow{g}{ti}") for ti in range(2)]
                srow_g[g] = srow
                for ti in range(2):
                    tq = workp.tile([8, 2 * NQ], f32, tag="ssqr", bufs=1,
                                    name=f"ssqr{g}{ti}")
                    nc.scalar.dma_start(out=tq[:, :ncols],
                                        in_=ssqg_out[g][:, ti, :])
                    red = workp.tile([8, 2 * NQ], f32, tag="ssqd", bufs=1,
                                     name=f"ssqd{g}{ti}")
                    nc.gpsimd.partition_all_reduce(
                        red[:, :ncols], tq[:, :ncols], channels=8,
                        reduce_op=bass_isa.ReduceOp.add)
                    dst = srow[ti][:, :ncols]
                    nc.scalar.activation(dst, red[:1, :ncols], Sqrt,
                                         scale=1.0 / D, bias=eps_col[:])
                    nc.vector.reciprocal_approx_fast(out=dst, in_=dst)

            def rope_j(j):
                qb, nn = N_SLICES[j]
                srow = srow_g[j // 2]
                col0 = (j % 2) * NQ
                for ti, dst in ((0, rq_sb[j]), (1, knew[j])):
                    srb = workp.tile([128, NQ], f32, tag="srowb", bufs=2)
                    nc.gpsimd.partition_broadcast(
                        srb[:, :nn], srow[ti][:, col0:col0 + nn])
                    cP = ftmp.tile([128, NQ], f32, tag="ropecs", bufs=2,
                                   name="ropec")
                    nc.vector.tensor_tensor(cP[:, :nn], cos_sb[:, qb:qb + nn],
                                            srb[:, :nn], mult_op)
                    sP = ftmp.tile([128, NQ], f32, tag="ropecs", bufs=2,
                                   name="ropes")
                    nc.vector.tensor_tensor(sP[:, :nn], sin_sb[:, qb:qb + nn],
                                            srb[:, :nn], mult_op)
                    for m in range(HPC):
                        st = dst[m][:, :nn]
                        sh = workp.tile([128, NQ], bf16, tag="btmp")
                        nc.vector.stream_shuffle(sh[:, :nn], st, SWAP_MASK)
                        a = ftmp.tile([128, NQ], f32, tag="f32tmp",
                                      name="rope_a")
                        nc.vector.tensor_tensor(a[:, :nn], st, cP[:, :nn],
                                                mult_op)
                        b = ftmp.tile([128, NQ], f32, tag="f32tmp",
                                      name="rope_b")
                        nc.vector.tensor_tensor(b[:, :nn], sh[:, :nn],
                                                sP[:, :nn], mult_op)
                        nc.vector.tensor_tensor(st, a[:, :nn], b[:, :nn],
                                                add_op)

            def vproj_j(j):
                qb, nn = N_SLICES[j]
                nst = (nn + 127) // 128
                xsg = []
                for g in range(KC // 4):
                    xs = xsp.tile([128, 4 * NQ], bf16, tag="xs", name="xsv")
                    nc.sync.dma_start(
                        out=xs[:].rearrange("p (a n) -> p a n", n=NQ)[:, :, :nn],
                        in_=xT[g * 512:(g + 1) * 512, qb:qb + nn]
                            .rearrange("(a p) n -> p a n", p=128))
                    xsg.append(xs)
                for s_ in range(nst):
                    sw = min(128, nn - s_ * 128)
                    pv = psac.tile([128, NQ], f32, tag="acc", name="pv_ps")
                    for g in range(KC // 4):
                        for kcl in range(4):
                            kc = g * 4 + kcl
                            nc.tensor.matmul(
                                pv[:sw, :MF],
                                xsg[g][:, kcl * NQ + s_ * 128:
                                       kcl * NQ + s_ * 128 + sw],
                                wslice_full("v", kc),
                                start=(kc == 0), stop=(kc == KC - 1))
                    for h in range(HPC):
                        nc.vector.tensor_copy(
                            vnew[j][h][:sw, s_ * 128:s_ * 128 + 128],
                            pv[:sw, h * 128:(h + 1) * 128])

            # ------------ attention + pipelined AG/y-proj ------
            scale = float(HD) ** -0.5

            def yproj(j):
                qb, nn = N_SLICES[j]
                py = [psac.tile([128, NQ], f32, tag="acc", name="py_ps")
                      for _ in range(HPC)]
                for g in range(KC // 4):
                    gt = xsp.tile([128, 4 * NQ], bf16, tag="xs", name="gt")
                    nc.sync.dma_start(
                        out=gt[:].rearrange("p (a n) -> p a n", n=NQ)[:, :, :nn],
                        in_=ag_out[j][:].rearrange("r m p n -> (r m p) n")
                            [g * 512:(g + 1) * 512, :]
                            .rearrange("(a p) n -> p a n", p=128))
                    for kcl in range(4):
                        kc = g * 4 + kcl
                        for m in range(HPC):
                            nc.tensor.matmul(
                                py[m][:, :nn], wslice("o", kc, m),
                                gt[:, kcl * NQ:kcl * NQ + nn],
                                start=(kc == 0), stop=(kc == KC - 1))
                for m in range(HPC):
                    ys = ftmp.tile([128, NQ], f32, tag="f32tmp", name="ys")
                    nc.scalar.copy(ys[:, :nn], py[m][:, :nn])
                    nc.gpsimd.dma_start(
                        out=yT[m * 128:(m + 1) * 128, qb:qb + nn],
                        in_=ys[:, :nn])

            def attn_j(j):
                qb, nn = N_SLICES[j]
                for h in range(HPC):
                    # tile list: (src_tile, col0, kt, vsrc, vcol, mask_off)
                    tiles = []
                    for ct in range(CTILES):
                        kt = min(128, CACHE - ct * 128)
                        tiles.append((kTc_sb[h], ct * 128, kt,
                                      vc_sb[h], ct * 128, None))
                    for t in range(NTILES):
                        kb = t * 128
                        if kb > qb + nn - 1:
                            continue
                        kt = min(128, S - kb)
                        jk, tl = kb // NQ, (kb % NQ) // 128
                        moff = (kb - qb) if (kb + kt - 1) > qb else None
                        tiles.append((knew[jk][h], tl * 128, kt,
                                      vnew[jk][h], tl * 128, moff))
                    pairs = []
                    i = 0
                    while i < len(tiles):
                        if (i + 1 < len(tiles) and tiles[i][2] == 128
                                and tiles[i + 1][2] == 128):
                            pairs.append((tiles[i], tiles[i + 1]))
                            i += 2
                        else:
                            pairs.append((tiles[i],))
                            i += 1
                    out_ps = psac.tile([128, NQ], f32, tag="acc",
                                       name="out_ps")
                    pacc = attnp.tile([128, 2 * NQ], f32, tag="pacc", bufs=2)
                    rq_slice = rq_sb[j][h][:, :nn]
                    nidx = 0
                    nlast = len(tiles) - 1
                    gacc = None
                    gcount = 0
                    pacc_init = False

                    def flush(nn=nn):
                        nonlocal gacc, gcount, pacc_init
                        if gacc is None:
                            return
                        gv = gacc[:].rearrange(
                            "p (a n) -> p a n", n=NQ)[:, :, :nn]
                        pv_ = pacc[:].rearrange(
                            "p (a n) -> p a n", n=NQ)[:, :, :nn]
                        if pacc_init:
                            nc.vector.tensor_tensor(pv_, pv_, gv, add_op)
                        else:
                            nc.vector.tensor_copy(pv_, gv)
                        gacc = None
                        gcount = 0
                        pacc_init = True

                    for pair in pairs:
                        full_pair = len(pair) == 2
                        sc = pssc.tile([128, 2 * NQ], f32, tag="scores")
                        for half, (src, c0, kt, vsrc, vcol, moff) in \
                                enumerate(pair):
                            nc.tensor.matmul(
                                sc[:kt, half * NQ:half * NQ + nn],
                                src[:, c0:c0 + kt],
                                rq_slice, start=True, stop=True)
                        pt = ptp.tile([128, 2 * NQ], bf16, tag="pT")
                        kt0 = pair[0][2]
                        if full_pair:
                            nc.scalar.activation(
                                pt[:].rearrange("p (a n) -> p a n",
                                                n=NQ)[:, :, :nn],
                                sc[:].rearrange("p (a n) -> p a n",
                                                n=NQ)[:, :, :nn],
                                Exp, scale=scale)
                        else:
                            nc.scalar.activation(pt[:kt0, :nn],
                                                 sc[:kt0, :nn], Exp,
                                                 scale=scale)
                        for half, (src, c0, kt, vsrc, vcol, moff) in \
                                enumerate(pair):
                            if moff is not None:
                                mi = moff // 128
                                nc.vector.tensor_tensor(
                                    pt[:kt, half * NQ:half * NQ + nn],
                                    pt[:kt, half * NQ:half * NQ + nn],
                                    mask_sb[:kt, mi * NQ:mi * NQ + nn],
                                    mult_op)
                        # denominator accumulation: bf16 groups of GSZ pairs,
                        # folded into fp32 pacc; odd tiles direct
                        if full_pair:
                            if gacc is None:
                                gacc = attnp.tile([128, 2 * NQ], bf16,
                                                  tag="gacc", bufs=2)
                                nc.vector.tensor_copy(
                                    gacc[:].rearrange("p (a n) -> p a n",
                                                      n=NQ)[:, :, :nn],
                                    pt[:].rearrange("p (a n) -> p a n",
                                                    n=NQ)[:, :, :nn])
                                gcount = 1
                            else:
                                nc.vector.tensor_tensor(
                                    gacc[:].rearrange("p (a n) -> p a n",
                                                      n=NQ)[:, :, :nn],
                                    gacc[:].rearrange("p (a n) -> p a n",
                                                      n=NQ)[:, :, :nn],
                                    pt[:].rearrange("p (a n) -> p a n",
                                                    n=NQ)[:, :, :nn],
                                    add_op)
                                gcount += 1
                            if gcount == GSZ:
                                flush()
                        else:
                            flush()
                            if pacc_init:
                                nc.vector.tensor_tensor(
                                    pacc[:kt0, :nn], pacc[:kt0, :nn],
                                    pt[:kt0, :nn], add_op)
                            else:
                                nc.vector.tensor_copy(pacc[:kt0, :nn],
                                                      pt[:kt0, :nn])
                                pacc_init = True
                        for half, (src, c0, kt, vsrc, vcol, moff) in \
                                enumerate(pair):
                            nc.tensor.matmul(
                                out_ps[:, :nn],
                                vsrc[:kt, vcol:vcol + 128],
                                pt[:kt, half * NQ:half * NQ + nn],
                                start=(nidx == 0), stop=(nidx == nlast))
                            nidx += 1
                    flush()
                    # fold the two halves, reduce over partitions, reciprocal
                    nc.vector.tensor_tensor(pacc[:, :nn], pacc[:, :nn],
                                            pacc[:, NQ:NQ + nn], add_op)
                    recb = attnp.tile([128, NQ], f32, tag="recb")
                    nc.gpsimd.partition_all_reduce(
                        recb[:, :nn], pacc[:, :nn], channels=128,
                        reduce_op=bass_isa.ReduceOp.add)
                    nc.vector.reciprocal_approx_fast(out=recb[:, :nn],
                                                     in_=recb[:, :nn])
                    onorm = attnp.tile([128, NQ], bf16, tag="onorm")
                    nc.vector.tensor_tensor(onorm[:, :nn], out_ps[:, :nn],
                                            recb[:, :nn], mult_op)
                    nc.gpsimd.dma_start(out=ag_in[j][h], in_=onorm[:, :nn])
                nc.gpsimd.collective_compute(
                    "AllGather", mybir.AluOpType.bypass, replica_groups=RG,
                    ins=[ag_in[j][:]], outs=[ag_out[j][:]])

            # ---------------- emission schedule ----------------
            qk_proj(0)
            nc.scalar.dma_start(out=cos_sb[:], in_=cosT[:])
            nc.scalar.dma_start(out=sin_sb[:], in_=sinT[:])
            bulk_loads()
            qk_proj(1)
            ssq_group(0)
            qk_proj(2)
            for c in range(WCH):
                nc.sync.dma_start(out=w_sb["v"][c][:], in_=wv[c])
            qk_proj(3)
            for c in range(WCH):
                nc.sync.dma_start(out=w_sb["o"][c][:], in_=wo[c])
            ssq_group_recv(0)
            rope_j(0)
            rope_j(1)
            vproj_j(0)
            attn_j(0)
            ssq_group(1)
            qk_proj(4)
            vproj_j(1)
            attn_j(1)
            yproj(0)
            qk_proj(5)
            ssq_group(2)
            ssq_group_recv(1)
            rope_j(2)
            vproj_j(2)
            attn_j(2)
            yproj(1)
            rope_j(3)
            ssq_group_recv(2)
            vproj_j(3)
            attn_j(3)
            yproj(2)
            rope_j(4)
            vproj_j(4)
            attn_j(4)
            yproj(3)
            rope_j(5)
            vproj_j(5)
            attn_j(5)
            yproj(4)
            yproj(5)
    nc.compile()
    return nc


def get_program():
    if "nc" not in _prog_cache:
        _prog_cache["nc"] = build_program()
    return _prog_cache["nc"]


def prep_inputs(x, freqs, k_cache, v_cache, Wq, bq, Wk, bk, Wv, bv, Wo, bo,
                gq, gk, current_start):
    """Host-side sharding/layout. Returns per-core in_maps."""
    cs = int(current_start)
    x = np.asarray(x, dtype=np.float32)
    xT = np.ascontiguousarray(x[0].T).astype(BF)           # [D, S]
    freqs = np.asarray(freqs, dtype=np.float32)
    csl = freqs[cs:cs + S, :HD // 2]                       # [S, 64]
    snl = freqs[cs:cs + S, HD // 2:]                       # [S, 64]
    cosT = np.empty((128, S), np.float32)
    sinT = np.empty((128, S), np.float32)
    cosT[0::2] = csl.T
    cosT[1::2] = csl.T
    sinT[0::2] = -snl.T
    sinT[1::2] = snl.T
    cosT = cosT.astype(BF)
    sinT = sinT.astype(BF)
    # spec guarantees zero biases and unit gains; the device program
    # relies on that (cheap to add back via K=1 bias matmuls if needed)
    for b in (bq, bk, bv, bo):
        assert not np.any(np.asarray(b)), "nonzero bias unsupported"
    for g in (gq, gk):
        assert np.all(np.asarray(g) == 1.0), "non-unit gain unsupported"
    # masks: multiplicative {0,1}, mask_d[r, c] = 1 if c >= r + d
    masks = np.zeros((4, 128, NQ), np.float32)
    r = np.arange(128)[:, None]
    c = np.arange(NQ)[None, :]
    for di, d in enumerate((0, 128, 256, 384)):
        masks[di] = (c >= r + d).astype(np.float32)
    masks = masks.astype(BF)

    k_cache = np.asarray(k_cache, np.float32)
    v_cache = np.asarray(v_cache, np.float32)

    def wlayout(W, sl, perm=None):
        wt = np.ascontiguousarray(np.asarray(W, np.float32)[sl].T)  # [D, MF]
        if perm is not None:
            wt = wt[perm]
        flat = np.ascontiguousarray(
            wt.reshape(KC, 128, MF).transpose(1, 0, 2)
            .reshape(128, KC * MF)).astype(BF)
        return np.ascontiguousarray(
            flat.reshape(128, WCH, 4 * MF).transpose(1, 0, 2))

    in_maps = []
    for core in range(N_CORES):
        h0 = core * HPC
        sl = slice(core * MF, (core + 1) * MF)
        kTcore = np.ascontiguousarray(
            np.transpose(k_cache[:, h0:h0 + HPC, :], (1, 2, 0))).astype(BF)
        vpad = np.zeros((HPC, 128, VPAD), BF)
        for h in range(HPC):
            vt = np.zeros((VPAD, HD), np.float32)
            vt[:CACHE] = v_cache[:, h0 + h, :]
            vpad[h] = np.ascontiguousarray(
                vt.reshape(CTILES, 128, HD).transpose(1, 0, 2)
                .reshape(128, VPAD)).astype(BF)
        in_maps.append({
            "xT": xT,
            "wq": wlayout(Wq, sl),
            "wk": wlayout(Wk, sl),
            "wv": wlayout(Wv, sl),
            "wo": wlayout(Wo, sl, perm=AG_PERM),
            "cosT": cosT,
            "sinT": sinT,
            "kTc": kTcore,
            "vc": vpad,
            "masks": masks,
        })
    return in_maps


def assemble_output(results):
    cols = [np.asarray(r["yT"], np.float32).T for r in results]  # [S, MF] each
    return np.ascontiguousarray(np.concatenate(cols, axis=1))[None]


def run(inputs, trace=False):
    nc = get_program()
    in_maps = prep_inputs(**inputs)
    r = run_bass_kernel_spmd(nc, in_maps, core_ids=list(range(N_CORES)),
                             trace=trace)
    return assemble_output(r.results), r


def kernel(**inputs):
    out, _ = run(inputs, trace=False)
    return out


# revision 30
# speedup vs baseline: 1.0417x; 1.0417x over previous
"""Trainium2 Bass kernel for nn_CausalSelfAttention_10368051052888.

Head-sharded tensor parallel over 8 NeuronCores (2 heads/core).
Feature-major ("transposed") layout on device: activations live as
[feature, seq] so the PE contraction dim is always the partition dim.

v2: software-pipelined per-slice schedule.
  - chunked weight DMAs so the first projection matmul starts ~6us in
  - ssq exchange as 3 small AllGathers (2 slices each) + local reduce,
    replacing the 2 serial latency-bound AllReduces
  - attention for slice j starts as soon as rope(j) is done; remaining
    qk/v projections and yproj fill PE while ACT runs the exp stream
  - one attention AllGather per slice (both heads) instead of per-head
  - per-slice staging tiles (rq/knew/vnew) for fine-grained deps

Self-contained: hardcodes the problem shapes from the spec.
"""
import numpy as np
import ml_dtypes

import concourse.bass as bass
import concourse.bass_isa as bass_isa
import concourse.mybir as mybir
import concourse.tile as tile
from concourse import bacc
from concourse.bass_utils import run_bass_kernel_spmd

BF = ml_dtypes.bfloat16

N_CORES = 8
S = 2640
D = 2048
H = 16
HD = 128
CACHE = 5280
EPS = 1e-6

HPC = H // N_CORES          # heads per core = 2
MF = HPC * HD               # per-core feature slice = 256
L = CACHE + S               # 7920
KC = D // 128               # 16 contraction chunks
WCH = 4                     # weight DMA chunks (4 kc each)
CTILES = (CACHE + 127) // 128   # 42 cache k-tiles (last kt=32)
NTILES = (S + 127) // 128       # 21 new k-tiles / v s-tiles (last 80)
VPAD = CTILES * 128             # 5376 padded cache rows for v
NQ = 512
N_SLICES = [(i * NQ, min(NQ, S - i * NQ)) for i in range((S + NQ - 1) // NQ)]
NJ = len(N_SLICES)
NG = 3                      # ssq exchange groups (2 slices each)
GSZ = 8                     # pairs per bf16 partial-sum group

SWAP_MASK = [(i ^ 1) for i in range(32)]  # pair swap within 32-partition groups

# wo contraction-row permutation: AG chunk layout is (core r, local head m,
# dim d) -> global feature (2r+m)*128+d
AG_PERM = np.array([(2 * (i // MF) + (i % MF) // HD) * HD + i % HD
                    for i in range(D)])

_prog_cache = {}


def build_program():
    dt = mybir.dt
    f32, bf16 = dt.float32, dt.bfloat16
    nc = bacc.Bacc("TRN2", target_bir_lowering=False, debug=False,
                   num_devices=N_CORES)

    # ---------------- I/O ----------------
    xT = nc.dram_tensor("xT", [D, S], bf16, kind="ExternalInput")
    wq = nc.dram_tensor("wq", [WCH, 128, 4 * MF], bf16, kind="ExternalInput")
    wk = nc.dram_tensor("wk", [WCH, 128, 4 * MF], bf16, kind="ExternalInput")
    wv = nc.dram_tensor("wv", [WCH, 128, 4 * MF], bf16, kind="ExternalInput")
    wo = nc.dram_tensor("wo", [WCH, 128, 4 * MF], bf16, kind="ExternalInput")
    cosT = nc.dram_tensor("cosT", [128, S], bf16, kind="ExternalInput")
    sinT = nc.dram_tensor("sinT", [128, S], bf16, kind="ExternalInput")
    kTc = nc.dram_tensor("kTc", [HPC, 128, CACHE], bf16, kind="ExternalInput")
    vc = nc.dram_tensor("vc", [HPC, 128, VPAD], bf16, kind="ExternalInput")
    masks = nc.dram_tensor("masks", [4, 128, NQ], bf16, kind="ExternalInput")
    yT = nc.dram_tensor("yT", [MF, S], f32, kind="ExternalOutput")

    ssqg_in = [nc.dram_tensor(f"ssqg_in{j}", [2, N_SLICES[j][1]], f32)
               for j in range(NJ)]
    ssqg_out = [nc.dram_tensor(f"ssqg_out{j}", [N_CORES, 2, N_SLICES[j][1]],
                               f32, addr_space="Shared") for j in range(NJ)]
    ag_in = [nc.dram_tensor(f"ag_in{j}", [HPC, 128, N_SLICES[j][1]], bf16)
             for j in range(NJ)]
    ag_out = [nc.dram_tensor(f"ag_out{j}",
                             [N_CORES, HPC, 128, N_SLICES[j][1]], bf16,
                             addr_space="Shared") for j in range(NJ)]

    RG = [list(range(N_CORES))]
    Exp = mybir.ActivationFunctionType.Exp
    Sqrt = mybir.ActivationFunctionType.Sqrt
    add_op = mybir.AluOpType.add
    mult_op = mybir.AluOpType.mult

    with tile.TileContext(nc) as tc:
        with (
            tc.tile_pool(name="const", bufs=1) as constp,
            tc.tile_pool(name="xs", bufs=4) as xsp,
            tc.tile_pool(name="work", bufs=3) as workp,
            tc.tile_pool(name="ftmp", bufs=3) as ftmp,
            tc.tile_pool(name="attn", bufs=2) as attnp,
            tc.tile_pool(name="ptp", bufs=3) as ptp,
            tc.tile_pool(name="psac", bufs=4, space="PSUM") as psac,
            tc.tile_pool(name="pssc", bufs=2, space="PSUM") as pssc,
        ):
            # ------------ persistent SBUF tiles ------------
            w_sb = {t: [constp.tile([128, 4 * MF], bf16, tag=f"w{t}{c}",
                                    name=f"w{t}{c}") for c in range(WCH)]
                    for t in ("q", "k", "v", "o")}

            def wslice(t, kc, m):
                c, kcl = kc // 4, kc % 4
                return w_sb[t][c][:, kcl * MF + m * 128:kcl * MF + (m + 1) * 128]

            def wslice_full(t, kc):
                c, kcl = kc // 4, kc % 4
                return w_sb[t][c][:, kcl * MF:(kcl + 1) * MF]

            cos_sb = constp.tile([128, S], bf16, tag="cos")
            sin_sb = constp.tile([128, S], bf16, tag="sin")
            mask_sb = constp.tile([128, 4 * NQ], bf16, tag="masks")
            kTc_sb = [constp.tile([128, CACHE], bf16, tag=f"kTc{h}",
                                  name=f"kTc{h}") for h in range(HPC)]
            vc_sb = [constp.tile([128, VPAD], bf16, tag=f"vc{h}",
                                 name=f"vc{h}") for h in range(HPC)]
            knew = [[constp.tile([128, N_SLICES[j][1]], bf16, tag=f"kn{j}{h}",
                                 name=f"kn{j}{h}") for h in range(HPC)]
                    for j in range(NJ)]
            rq_sb = [[constp.tile([128, N_SLICES[j][1]], bf16, tag=f"rq{j}{h}",
                                  name=f"rq{j}{h}") for h in range(HPC)]
                     for j in range(NJ)]
            vnew = [[constp.tile([128, ((N_SLICES[j][1] + 127) // 128) * 128],
                                 bf16, tag=f"vn{j}{h}",
                                 name=f"vn{j}{h}") for h in range(HPC)]
                    for j in range(NJ)]
            onescol = constp.tile([128, 1], bf16, tag="onescol")
            nc.vector.memset(onescol[:], 1.0)
            ones8 = constp.tile([8, 1], f32, tag="ones8")
            nc.vector.memset(ones8[:], 1.0)
            eps_col = constp.tile([1, 1], f32, tag="eps")
            nc.vector.memset(eps_col[:], EPS)

            # ------------ prologue DMAs ------------
            # weights chunked and interleaved with the first slice's xs
            # stream so the first projection matmul only waits ~1MB of DMA.
            # Bulk cache/v/mask loads are emitted later, gated behind the
            # first slice's staging so their descriptors can't head-of-line
            # block the xT stream at t=0.
            def bulk_loads():
                # dummy dep: don't start bulk until slice 0 is staged
                gate = workp.tile([128, 1], bf16, tag="gate", bufs=1)
                nc.gpsimd.partition_broadcast(gate[:], rq_sb[0][0][0:1, 0:1])
                for h in range(HPC):
                    nc.gpsimd.dma_start(out=kTc_sb[h][:], in_=kTc[h])
                for h in range(HPC):
                    nc.gpsimd.dma_start(out=vc_sb[h][:], in_=vc[h])
                nc.gpsimd.dma_start(
                    out=mask_sb[:].rearrange("p (d c) -> p d c", c=NQ),
                    in_=masks[:].rearrange("d p c -> p d c"),
                )

            def stream_x(qb, nn, consume, wdma=None):
                """DMA xT[:, qb:qb+nn] in 4-chunk groups; call consume(kc, rhs_ap)."""
                for g in range(KC // 4):
                    if wdma is not None:
                        wdma(g)
                    xs = xsp.tile([128, 4 * NQ], bf16, tag="xs", name="xs")
                    nc.sync.dma_start(
                        out=xs[:].rearrange("p (a n) -> p a n", n=NQ)[:, :, :nn],
                        in_=xT[g * 512:(g + 1) * 512, qb:qb + nn]
                            .rearrange("(a p) n -> p a n", p=128))
                    for kcl in range(4):
                        consume(g * 4 + kcl, xs[:, kcl * NQ:kcl * NQ + nn])

            def qk_proj(j):
                """Projection + rope (norm-preserving, so ssq comes after)
                + ssq partials + per-slice ssq AllGather. None of this
                depends on a collective, so the whole slice pipeline runs
                back-to-back; only the tiny srow scale-mult (applied in
                srow_recv) waits for the gather."""
                qb, nn = N_SLICES[j]
                pst = {t: [psac.tile([128, NQ], f32, tag="acc",
                                     name=f"proj_{t}{m}")
                           for m in range(HPC)] for t in ("q", "k")}

                def mm_proj(kc, rhs, pst=pst, nn=nn):
                    for t in ("q", "k"):
                        for m in range(HPC):
                            nc.tensor.matmul(
                                pst[t][m][:, :nn], wslice(t, kc, m),
                                rhs, start=(kc == 0), stop=(kc == KC - 1))

                def wdma(g):
                    nc.sync.dma_start(out=w_sb["q"][g][:], in_=wq[g])
                    nc.sync.dma_start(out=w_sb["k"][g][:], in_=wk[g])

                stream_x(qb, nn, mm_proj, wdma=wdma if j == 0 else None)
                # stage raw q/k as bf16 (ACT)
                for t, dst in (("q", rq_sb[j]), ("k", knew[j])):
                    for m in range(HPC):
                        nc.scalar.copy(dst[m][:, :nn], pst[t][m][:, :nn])
                # rope in place (no srow yet)
                for dst in (rq_sb[j], knew[j]):
                    for m in range(HPC):
                        st = dst[m][:, :nn]
                        sh = workp.tile([128, NQ], bf16, tag="btmp")
                        nc.vector.stream_shuffle(sh[:, :nn], st, SWAP_MASK)
                        a = ftmp.tile([128, NQ], f32, tag="f32tmp",
                                      name="rope_a")
                        nc.vector.tensor_tensor(a[:, :nn], st,
                                                cos_sb[:, qb:qb + nn],
                                                mult_op)
                        b = ftmp.tile([128, NQ], f32, tag="f32tmp",
                                      name="rope_b")
                        nc.vector.tensor_tensor(b[:, :nn], sh[:, :nn],
                                                sin_sb[:, qb:qb + nn],
                                                mult_op)
                        nc.vector.tensor_tensor(st, a[:, :nn], b[:, :nn],
                                                add_op)
                # ssq partials (rope is norm-preserving) + AllGather
                parts = []
                for ti, src in enumerate((rq_sb[j], knew[j])):
                    sqp = pssc.tile([128, 2 * NQ], f32, tag="scores",
                                    name="sqp")
                    for m in range(HPC):
                        q2 = workp.tile([128, NQ], bf16, tag="btmp")
                        nc.vector.tensor_tensor(q2[:, :nn], src[m][:, :nn],
                                                src[m][:, :nn], mult_op)
                        nc.tensor.matmul(sqp[:1, :nn], onescol[:],
                                         q2[:, :nn],
                                         start=(m == 0), stop=(m == HPC - 1))
                    ptile = workp.tile([1, NQ], f32, tag=f"part{ti}", bufs=2,
                                       name=f"part{j}{ti}")
                    nc.scalar.copy(ptile[:, :nn], sqp[:1, :nn])
                    parts.append(ptile)
                for ti in range(2):
                    nc.scalar.dma_start(out=ssqg_in[j][ti:ti + 1, :],
                                        in_=parts[ti][:, :nn])
                nc.gpsimd.collective_compute(
                    "AllGather", mybir.AluOpType.bypass, replica_groups=RG,
                    ins=[ssqg_in[j][:]], outs=[ssqg_out[j][:]])

            def srow_recv(j):
                """Reduce gathered ssq partials; scale rq/knew by srow."""
                qb, nn = N_SLICES[j]
                tq = workp.tile([8, 2 * NQ], f32, tag="ssqr", bufs=2,
                                name=f"ssqr{j}")
                nc.scalar.dma_start(
                    out=tq[:].rearrange("p (t n) -> p t n", t=2)[:, :, :nn],
                    in_=ssqg_out[j][:])
                red = pssc.tile([128, 2 * NQ], f32, tag="scores",
                                name=f"ssqd{j}")
                for half in range(2):
                    nc.tensor.matmul(
                        red[:1, half * NQ:half * NQ + nn], ones8[:],
                        tq[:, half * NQ:half * NQ + nn],
                        start=True, stop=True)
                srow = workp.tile([1, 2 * NQ], f32, tag="srowg", bufs=2,
                                  name=f"srow{j}")
                for half in range(2):
                    sl = slice(half * NQ, half * NQ + nn)
                    nc.scalar.activation(srow[:, sl], red[:1, sl],
                                         Sqrt, scale=1.0 / D, bias=eps_col[:])
                    nc.vector.reciprocal_approx_fast(out=srow[:, sl],
                                                     in_=srow[:, sl])
                for ti, dst in ((0, rq_sb[j]), (1, knew[j])):
                    srb = workp.tile([128, NQ], f32, tag="srowb", bufs=2)
                    nc.gpsimd.partition_broadcast(
                        srb[:, :nn], srow[:, ti * NQ:ti * NQ + nn])
                    for m in range(HPC):
                        nc.vector.tensor_tensor(dst[m][:, :nn],
                                                dst[m][:, :nn],
                                                srb[:, :nn], mult_op)

            def vproj_j(j):
                qb, nn = N_SLICES[j]
                nst = (nn + 127) // 128
                xsg = []
                for g in range(KC // 4):
                    xs = xsp.tile([128, 4 * NQ], bf16, tag="xs", name="xsv")
                    nc.sync.dma_start(
                        out=xs[:].rearrange("p (a n) -> p a n", n=NQ)[:, :, :nn],
                        in_=xT[g * 512:(g + 1) * 512, qb:qb + nn]
                            .rearrange("(a p) n -> p a n", p=128))
                    xsg.append(xs)
                for s_ in range(nst):
                    sw = min(128, nn - s_ * 128)
                    pv = psac.tile([128, NQ], f32, tag="acc", name="pv_ps")
                    for g in range(KC // 4):
                        for kcl in range(4):
                            kc = g * 4 + kcl
                            nc.tensor.matmul(
                                pv[:sw, :MF],
                                xsg[g][:, kcl * NQ + s_ * 128:
                                       kcl * NQ + s_ * 128 + sw],
                                wslice_full("v", kc),
                                start=(kc == 0), stop=(kc == KC - 1))
                    for h in range(HPC):
                        nc.vector.tensor_copy(
                            vnew[j][h][:sw, s_ * 128:s_ * 128 + 128],
                            pv[:sw, h * 128:(h + 1) * 128])

            # ------------ attention + pipelined AG/y-proj ------
            scale = float(HD) ** -0.5

            def yproj(j):
                qb, nn = N_SLICES[j]
                py = [psac.tile([128, NQ], f32, tag="acc", name="py_ps")
                      for _ in range(HPC)]
                for g in range(KC // 4):
                    gt = xsp.tile([128, 4 * NQ], bf16, tag="xs", name="gt")
                    nc.sync.dma_start(
                        out=gt[:].rearrange("p (a n) -> p a n", n=NQ)[:, :, :nn],
                        in_=ag_out[j][:].rearrange("r m p n -> (r m p) n")
                            [g * 512:(g + 1) * 512, :]
                            .rearrange("(a p) n -> p a n", p=128))
                    for kcl in range(4):
                        kc = g * 4 + kcl
                        for m in range(HPC):
                            nc.tensor.matmul(
                                py[m][:, :nn], wslice("o", kc, m),
                                gt[:, kcl * NQ:kcl * NQ + nn],
                                start=(kc == 0), stop=(kc == KC - 1))
                for m in range(HPC):
                    ys = ftmp.tile([128, NQ], f32, tag="f32tmp", name="ys")
                    nc.scalar.copy(ys[:, :nn], py[m][:, :nn])
                    nc.gpsimd.dma_start(
                        out=yT[m * 128:(m + 1) * 128, qb:qb + nn],
                        in_=ys[:, :nn])

            def attn_j(j):
                qb, nn = N_SLICES[j]
                for h in range(HPC):
                    # tile list: (src_tile, col0, kt, vsrc, vcol, mask_off)
                    tiles = []
                    for ct in range(CTILES):
                        kt = min(128, CACHE - ct * 128)
                        tiles.append((kTc_sb[h], ct * 128, kt,
                                      vc_sb[h], ct * 128, None))
                    for t in range(NTILES):
                        kb = t * 128
                        if kb > qb + nn - 1:
                            continue
                        kt = min(128, S - kb)
                        jk, tl = kb // NQ, (kb % NQ) // 128
                        moff = (kb - qb) if (kb + kt - 1) > qb else None
                        tiles.append((knew[jk][h], tl * 128, kt,
                                      vnew[jk][h], tl * 128, moff))
                    pairs = []
                    i = 0
                    while i < len(tiles):
                        if (i + 1 < len(tiles) and tiles[i][2] == 128
                                and tiles[i + 1][2] == 128):
                            pairs.append((tiles[i], tiles[i + 1]))
                            i += 2
                        else:
                            pairs.append((tiles[i],))
                            i += 1
                    out_ps = psac.tile([128, NQ], f32, tag="acc",
                                       name="out_ps")
                    pacc = attnp.tile([128, 2 * NQ], f32, tag="pacc", bufs=2)
                    rq_slice = rq_sb[j][h][:, :nn]
                    nidx = 0
                    nlast = len(tiles) - 1
                    gacc = None
                    gcount = 0
                    pacc_init = False

                    def flush(nn=nn):
                        nonlocal gacc, gcount, pacc_init
                        if gacc is None:
                            return
                        gv = gacc[:].rearrange(
                            "p (a n) -> p a n", n=NQ)[:, :, :nn]
                        pv_ = pacc[:].rearrange(
                            "p (a n) -> p a n", n=NQ)[:, :, :nn]
                        if pacc_init:
                            nc.vector.tensor_tensor(pv_, pv_, gv, add_op)
                        else:
                            nc.vector.tensor_copy(pv_, gv)
                        gacc = None
                        gcount = 0
                        pacc_init = True

                    for pair in pairs:
                        full_pair = len(pair) == 2
                        sc = pssc.tile([128, 2 * NQ], f32, tag="scores")
                        for half, (src, c0, kt, vsrc, vcol, moff) in \
                                enumerate(pair):
                            nc.tensor.matmul(
                                sc[:kt, half * NQ:half * NQ + nn],
                                src[:, c0:c0 + kt],
                                rq_slice, start=True, stop=True)
                        pt = ptp.tile([128, 2 * NQ], bf16, tag="pT")
                        kt0 = pair[0][2]
                        if full_pair:
                            nc.scalar.activation(
                                pt[:].rearrange("p (a n) -> p a n",
                                                n=NQ)[:, :, :nn],
                                sc[:].rearrange("p (a n) -> p a n",
                                                n=NQ)[:, :, :nn],
                                Exp, scale=scale)
                        else:
                            nc.scalar.activation(pt[:kt0, :nn],
                                                 sc[:kt0, :nn], Exp,
                                                 scale=scale)
                        for half, (src, c0, kt, vsrc, vcol, moff) in \
                                enumerate(pair):
                            if moff is not None:
                                mi = moff // 128
                                nc.vector.tensor_tensor(
                                    pt[:kt, half * NQ:half * NQ + nn],
                                    pt[:kt, half * NQ:half * NQ + nn],
                                    mask_sb[:kt, mi * NQ:mi * NQ + nn],
                                    mult_op)
                        # denominator accumulation: bf16 groups of GSZ pairs,
                        # folded into fp32 pacc; odd tiles direct
                        if full_pair:
                            if gacc is None:
                                gacc = attnp.tile([128, 2 * NQ], bf16,
                                                  tag="gacc", bufs=2)
                                nc.vector.tensor_copy(
                                    gacc[:].rearrange("p (a n) -> p a n",
                                                      n=NQ)[:, :, :nn],
                                    pt[:].rearrange("p (a n) -> p a n",
                                                    n=NQ)[:, :, :nn])
                                gcount = 1
                            else:
                                nc.vector.tensor_tensor(
                                    gacc[:].rearrange("p (a n) -> p a n",
                                                      n=NQ)[:, :, :nn],
                                    gacc[:].rearrange("p (a n) -> p a n",
                                                      n=NQ)[:, :, :nn],
                                    pt[:].rearrange("p (a n) -> p a n",
                                                    n=NQ)[:, :, :nn],
                                    add_op)
                                gcount += 1
                            if gcount == GSZ:
                                flush()
                        else:
                            flush()
                            if pacc_init:
                                nc.vector.tensor_tensor(
                                    pacc[:kt0, :nn], pacc[:kt0, :nn],
                                    pt[:kt0, :nn], add_op)
                            else:
                                nc.vector.tensor_copy(pacc[:kt0, :nn],
                                                      pt[:kt0, :nn])
                                pacc_init = True
                        for half, (src, c0, kt, vsrc, vcol, moff) in \
                                enumerate(pair):
                            nc.tensor.matmul(
                                out_ps[:, :nn],
                                vsrc[:kt, vcol:vcol + 128],
                                pt[:kt, half * NQ:half * NQ + nn],
                                start=(nidx == 0), stop=(nidx == nlast))
                            nidx += 1
                    flush()
                    # fold the two halves, reduce over partitions, reciprocal
                    nc.vector.tensor_tensor(pacc[:, :nn], pacc[:, :nn],
                                            pacc[:, NQ:NQ + nn], add_op)
                    recb = attnp.tile([128, NQ], f32, tag="recb")
                    nc.gpsimd.partition_all_reduce(
                        recb[:, :nn], pacc[:, :nn], channels=128,
                        reduce_op=bass_isa.ReduceOp.add)
                    nc.vector.reciprocal_approx_fast(out=recb[:, :nn],
                                                     in_=recb[:, :nn])
                    onorm = attnp.tile([128, NQ], bf16, tag="onorm")
                    nc.vector.tensor_tensor(onorm[:, :nn], out_ps[:, :nn],
                                            recb[:, :nn], mult_op)
                    nc.gpsimd.dma_start(out=ag_in[j][h], in_=onorm[:, :nn])
                nc.gpsimd.collective_compute(
                    "AllGather", mybir.AluOpType.bypass, replica_groups=RG,
                    ins=[ag_in[j][:]], outs=[ag_out[j][:]])

            # ---------------- emission schedule ----------------
            nc.scalar.dma_start(out=cos_sb[:], in_=cosT[:])
            nc.scalar.dma_start(out=sin_sb[:], in_=sinT[:])
            qk_proj(0)
            bulk_loads()
            qk_proj(1)
            qk_proj(2)
            for c in range(WCH):
                nc.sync.dma_start(out=w_sb["v"][c][:], in_=wv[c])
            qk_proj(3)
            for c in range(WCH):
                nc.sync.dma_start(out=w_sb["o"][c][:], in_=wo[c])
            vproj_j(0)
            srow_recv(0)
            attn_j(0)
            qk_proj(4)
            vproj_j(1)
            srow_recv(1)
            attn_j(1)
            yproj(0)
            qk_proj(5)
            vproj_j(2)
            srow_recv(2)
            attn_j(2)
            yproj(1)
            vproj_j(3)
            srow_recv(3)
            attn_j(3)
            yproj(2)
            vproj_j(4)
            srow_recv(4)
            attn_j(4)
            yproj(3)
            vproj_j(5)
            srow_recv(5)
            attn_j(5)
            yproj(4)
            yproj(5)
    nc.compile()
    return nc


def get_program():
    if "nc" not in _prog_cache:
        _prog_cache["nc"] = build_program()
    return _prog_cache["nc"]


def prep_inputs(x, freqs, k_cache, v_cache, Wq, bq, Wk, bk, Wv, bv, Wo, bo,
                gq, gk, current_start):
    """Host-side sharding/layout. Returns per-core in_maps."""
    cs = int(current_start)
    x = np.asarray(x, dtype=np.float32)
    xT = np.ascontiguousarray(x[0].T).astype(BF)           # [D, S]
    freqs = np.asarray(freqs, dtype=np.float32)
    csl = freqs[cs:cs + S, :HD // 2]                       # [S, 64]
    snl = freqs[cs:cs + S, HD // 2:]                       # [S, 64]
    cosT = np.empty((128, S), np.float32)
    sinT = np.empty((128, S), np.float32)
    cosT[0::2] = csl.T
    cosT[1::2] = csl.T
    sinT[0::2] = -snl.T
    sinT[1::2] = snl.T
    cosT = cosT.astype(BF)
    sinT = sinT.astype(BF)
    # spec guarantees zero biases and unit gains; the device program
    # relies on that (cheap to add back via K=1 bias matmuls if needed)
    for b in (bq, bk, bv, bo):
        assert not np.any(np.asarray(b)), "nonzero bias unsupported"
    for g in (gq, gk):
        assert np.all(np.asarray(g) == 1.0), "non-unit gain unsupported"
    # masks: multiplicative {0,1}, mask_d[r, c] = 1 if c >= r + d
    masks = np.zeros((4, 128, NQ), np.float32)
    r = np.arange(128)[:, None]
    c = np.arange(NQ)[None, :]
    for di, d in enumerate((0, 128, 256, 384)):
        masks[di] = (c >= r + d).astype(np.float32)
    masks = masks.astype(BF)

    k_cache = np.asarray(k_cache, np.float32)
    v_cache = np.asarray(v_cache, np.float32)

    def wlayout(W, sl, perm=None):
        wt = np.ascontiguousarray(np.asarray(W, np.float32)[sl].T)  # [D, MF]
        if perm is not None:
            wt = wt[perm]
        flat = np.ascontiguousarray(
            wt.reshape(KC, 128, MF).transpose(1, 0, 2)
            .reshape(128, KC * MF)).astype(BF)
        return np.ascontiguousarray(
            flat.reshape(128, WCH, 4 * MF).transpose(1, 0, 2))

    in_maps = []
    for core in range(N_CORES):
        h0 = core * HPC
        sl = slice(core * MF, (core + 1) * MF)
        kTcore = np.ascontiguousarray(
            np.transpose(k_cache[:, h0:h0 + HPC, :], (1, 2, 0))).astype(BF)
        vpad = np.zeros((HPC, 128, VPAD), BF)
        for h in range(HPC):
            vt = np.zeros((VPAD, HD), np.float32)
            vt[:CACHE] = v_cache[:, h0 + h, :]
            vpad[h] = np.ascontiguousarray(
                vt.reshape(CTILES, 128, HD).transpose(1, 0, 2)
                .reshape(128, VPAD)).astype(BF)
        in_maps.append({
            "xT": xT,
            "wq": wlayout(Wq, sl),
            "wk": wlayout(Wk, sl),
            "wv": wlayout(Wv, sl),
            "wo": wlayout(Wo, sl, perm=AG_PERM),
            "cosT": cosT,
            "sinT": sinT,
            "kTc": kTcore,
            "vc": vpad,
            "masks": masks,
        })
    return in_maps


def assemble_output(results):
    cols = [np.asarray(r["yT"], np.float32).T for r in results]  # [S, MF] each
    return np.ascontiguousarray(np.concatenate(cols, axis=1))[None]


def run(inputs, trace=False):
    nc = get_program()
    in_maps = prep_inputs(**inputs)
    r = run_bass_kernel_spmd(nc, in_maps, core_ids=list(range(N_CORES)),
                             trace=trace)
    return assemble_output(r.results), r


def kernel(**inputs):
    out, _ = run(inputs, trace=False)
    return out
